# revision 44
# baseline (speedup 1.0000x reference)
"""Trainium2 Bass kernel for nn_Attention_xxc (dense transformer attention
with hop-distance bias). Data-parallel over batch: 8 cores x 2 batches.

Wire-traffic-minimized design: the warm end-to-end latency of this problem
is dominated by host<->device transfer over the axon tunnel (~50 MB/s), so
every shared tensor is shipped sharded 1/8-per-core and AllGathered on
device over NeuronLink; the hop-bias mixture  alpha_h * sum_k w_hk Hstack_k
is never materialized on the host - the PE folds it into the score matmuls
as  S.T = K^T Q + sum_k (c_hk I) @ Hstack_k.T  accumulated in PSUM.

Per-core layout (core c of 8):
  - xn [2048, 512] bf16: the core's own 2 batches, natural layout; the PE
    transposes it on device via identity matmuls.
  - shards (rows c/8) of: HTs flat [5120,1024] (Hstack_k transposed),
    wqkvT [512,1536] (q cols pre-scaled 1/sqrt(hd)), wprojT [512,512],
    ceye flat [5120,128] (40 scaled identities c_hk*I), eye128.
  - qkv: q,k TRANSPOSED ([outch, tok] bf16), v NATURAL with a ones column
    per head (65 cols/head) so the AV matmul also produces the softmax
    denominator in row 64.
  - output y [2048, 512] bf16, host casts to f32.
Runner: persistent jax jit of the bass_exec custom call (no per-call
retrace), donated output buffers are created on device (no host zeros).
"""
import sys

sys.path.insert(0, "/opt/trn_rl_repo")

import numpy as np
import ml_dtypes

B, N, DIM = 16, 1024, 512
H, HD, KH = 8, 64, 5
SCALE = HD ** -0.5
NCORES = 8
BPC = B // NCORES          # batches per core
TOK = BPC * N              # tokens per core = 2048
HTR = KH * N               # 5120 rows of flat transposed-Hstack
CER = H * KH * 128         # 5120 rows of flat scaled-identity stack

# shared-blob layout, in rows of 1024 bytes (= 512 bf16 / 1024 u8):
#   wqkv bf16 [512,1536] | wproj bf16 [512,512] | ceye bf16 [5120,128]
#   | eye bf16 [128,128] | bproj bf16 [512] | hts u8 [5120,1024] | pad
R_WQKV = 0
R_WPROJ = R_WQKV + 512 * 3
R_CEYE = R_WPROJ + 512
R_EYE = R_CEYE + CER // 4
R_BPROJ = R_EYE + 32
R_HTS = R_BPROJ + 1
SHR_ROWS = -(-(R_HTS + HTR) // 8) * 8    # pad to a multiple of 8 cores

_CACHE = {}


def _build():
    import concourse.bass as bass
    import concourse.bacc as bacc
    import concourse.mybir as mybir
    from concourse.tile import TileContext

    f32 = mybir.dt.float32
    bf16 = mybir.dt.bfloat16
    u8 = mybir.dt.uint8
    f8 = mybir.dt.float8e4
    EXP = mybir.ActivationFunctionType.Exp
    MUL = mybir.AluOpType.mult
    ADD = mybir.AluOpType.add
    BYP = mybir.AluOpType.bypass
    RG = [list(range(NCORES))]

    nc = bacc.Bacc(num_devices=NCORES)
    xn = nc.declare_dram_parameter("xn", [TOK, DIM], f8, isOutput=False)
    shr_in = nc.declare_dram_parameter("shr_in", [SHR_ROWS // 8, 1024], u8, isOutput=False)
    y = nc.declare_dram_parameter("y", [TOK, DIM], bf16, isOutput=True)

    NT = TOK // 128            # 16 token tiles per core
    VW = H * (HD + 1)          # 520: v row width with ones col per head

    with TileContext(nc) as tc:
        with (
            tc.tile_pool(name="dram", bufs=1, space="DRAM") as DR,
            tc.tile_pool(name="qk", bufs=1) as QK,
            tc.tile_pool(name="vres", bufs=1) as VR,
            tc.tile_pool(name="wp", bufs=1) as WP,
            tc.tile_pool(name="outT", bufs=1) as OT,
            tc.tile_pool(name="const", bufs=1) as CONST,
        ):
            # ---------------- phase 0: AllGather the one shared blob ----------------
            bnc = DR.tile([SHR_ROWS // 8, 1024], u8, tag="b_shr", name="b_shr")
            shr_full = DR.tile([SHR_ROWS, 1024], u8, tag="g_shr", name="g_shr")
            nc.gpsimd.dma_start(bnc[:], shr_in[:])
            nc.gpsimd.collective_compute(
                "AllGather", BYP, replica_groups=RG,
                ins=[bnc.opt()], outs=[shr_full.opt()])

            eye_t = CONST.tile([128, 128], bf16, tag="eye", name="eye")
            nc.sync.dma_start(
                out=eye_t[:],
                in_=shr_full[R_EYE: R_EYE + 32, :].bitcast(bf16)
                .rearrange("a (b c) -> (a b) c", b=4))
            ones_t = CONST.tile([1, 128], bf16, tag="ones", name="ones")
            nc.vector.memset(ones_t[:], 1.0)
            ceye_t = CONST.tile([128, H * KH * 128], bf16, tag="ceye", name="ceye")
            for j in range(H * KH):
                nc.sync.dma_start(
                    out=ceye_t[:, j * 128:(j + 1) * 128],
                    in_=shr_full[R_CEYE + 32 * j: R_CEYE + 32 * (j + 1), :]
                    .bitcast(bf16).rearrange("a (b c) -> (a b) c", b=4))
            wp_t = [WP.tile([128, DIM], bf16, tag=f"wp{c}", name=f"wp{c}") for c in range(4)]
            for c in range(4):
                nc.sync.dma_start(
                    out=wp_t[c][:],
                    in_=shr_full[R_WPROJ + c * 128: R_WPROJ + (c + 1) * 128, :]
                    .bitcast(bf16))

            qk_t = [QK.tile([128, TOK], bf16, tag=f"qk{o}", name=f"qk{o}") for o in range(8)]
            v_t = [VR.tile([128, VW], bf16, tag=f"v{t}", name=f"v{t}") for t in range(NT)]
            oT_t = [OT.tile([128, N], bf16, tag=f"oT{b}_{c}", name=f"oT{b}_{c}")
                    for b in range(BPC) for c in range(4)]

            # broadcast bproj across 128 partitions: ones^T [128] x bproj [1,512]
            bpb_t = CONST.tile([128, DIM], f32, tag="bpb", name="bpb")
            bpr_t = CONST.tile([1, DIM], bf16, tag="bpr", name="bpr")
            nc.sync.dma_start(out=bpr_t[:],
                              in_=shr_full[R_BPROJ: R_BPROJ + 1, :].bitcast(bf16))

            # ---------------- phase 1: x transpose + qkv projections ----------------
            with (
                tc.tile_pool(name="xw", bufs=1) as XW,
                tc.tile_pool(name="ps1", bufs=4, space="PSUM") as PS1,
                tc.tile_pool(name="pst", bufs=4, space="PSUM") as PST,
            ):
                psb = PS1.tile([128, DIM], f32, tag="ps1", name="ps1")
                nc.tensor.matmul(psb[:], ones_t[:], bpr_t[:], start=True, stop=True)
                nc.vector.tensor_copy(bpb_t[:], psb[:])

                xn_t = [XW.tile([128, DIM], bf16, tag=f"xn{t}", name=f"xn{t}")
                        for t in range(NT)]
                for t in range(NT):
                    x8 = XW.tile([128, DIM], f8, tag=f"x8_{t}", name=f"x8_{t}")
                    nc.sync.dma_start(out=x8[:], in_=xn[t * 128:(t + 1) * 128, :])
                    nc.vector.tensor_copy(xn_t[t][:], x8[:])
                xT_t = [XW.tile([128, TOK], bf16, tag=f"x{c}", name=f"x{c}") for c in range(4)]
                for t in range(NT):
                    for c in range(4):
                        pst = PST.tile([128, 128], f32, tag="pst", name="pst")
                        nc.tensor.matmul(pst[:], xn_t[t][:, c * 128:(c + 1) * 128],
                                         eye_t[:], start=True, stop=True)
                        nc.vector.tensor_copy(xT_t[c][:, t * 128:(t + 1) * 128], pst[:])

                wq_t = [XW.tile([128, 3 * DIM], bf16, tag=f"w{c}", name=f"w{c}") for c in range(4)]
                for c in range(4):
                    for t in range(3):
                        nc.sync.dma_start(
                            out=wq_t[c][:, 512 * t:512 * (t + 1)],
                            in_=shr_full[R_WQKV + 384 * c + t:
                                         R_WQKV + 384 * (c + 1): 3, :].bitcast(bf16))

                # q,k transposed: qkvT[o_tile, tok] ; o tiles 0..7 cover q,k
                for o in range(8):
                    for t in range(4):           # tok chunks of 512
                        ps = PS1.tile([128, 512], f32, tag="ps1", name="ps1")
                        for c in range(4):
                            nc.tensor.matmul(
                                ps[:], wq_t[c][:, o * 128:(o + 1) * 128],
                                xT_t[c][:, t * 512:(t + 1) * 512],
                                start=(c == 0), stop=(c == 3))
                        nc.vector.tensor_copy(qk_t[o][:, t * 512:(t + 1) * 512], ps[:])
                # v natural: [tok_tile, vch] -> packed per head with ones col
                for t in range(NT):
                    ps = PS1.tile([128, 512], f32, tag="ps1", name="ps1")
                    for c in range(4):
                        nc.tensor.matmul(
                            ps[:], xT_t[c][:, t * 128:(t + 1) * 128],
                            wq_t[c][:, 2 * DIM:3 * DIM],
                            start=(c == 0), stop=(c == 3))
                    dst = v_t[t][:, 0:VW].rearrange("p (h s) -> p h s", s=HD + 1)
                    nc.vector.tensor_copy(
                        dst[:, :, 0:HD],
                        ps[:].rearrange("p (h s) -> p h s", s=HD))
                    nc.vector.memset(dst[:, :, HD:HD + 1], 1.0)

            # ---------------- phase 2: attention ----------------
            with (
                tc.tile_pool(name="htu", bufs=3) as HTU,
                tc.tile_pool(name="htp", bufs=4) as HTP,
                tc.tile_pool(name="pp", bufs=18) as PP,
                tc.tile_pool(name="nrm", bufs=4) as NRM,
                tc.tile_pool(name="ysb", bufs=3) as YSB,
                tc.tile_pool(name="pss", bufs=2, space="PSUM") as PSS,
                tc.tile_pool(name="pso", bufs=1, space="PSUM") as PSO,
                tc.tile_pool(name="psm", bufs=2, space="PSUM") as PSM,
            ):
                for h in range(H):
                    qt, po = qk_t[h // 2], (h % 2) * 64
                    kt = qk_t[4 + h // 2]
                    p_tiles = [[], []]
                    for mi in range(8):
                        hu = HTU.tile([128, KH * N], u8, tag="hu", name="hu")
                        for k in range(KH):
                            nc.sync.dma_start(
                                out=hu[:, k * N:(k + 1) * N],
                                in_=shr_full[R_HTS + k * N + mi * 128:
                                             R_HTS + k * N + (mi + 1) * 128, :])
                        ht = HTP.tile([128, KH * N], bf16, tag="ht", name="ht")
                        nc.vector.tensor_copy(ht[:], hu[:])
                        for b in range(BPC):
                            t0 = b * N
                            ps = PSS.tile([128, N], f32, tag="pss", name="pss")
                            for nchunk in range(2):
                                sl = slice(nchunk * 512, (nchunk + 1) * 512)
                                nc.tensor.matmul(
                                    ps[:, sl],
                                    kt[po:po + 64, t0 + mi * 128: t0 + (mi + 1) * 128],
                                    qt[po:po + 64, t0 + nchunk * 512: t0 + (nchunk + 1) * 512],
                                    start=True, stop=False)
                                for k in range(KH):
                                    ci = (h * KH + k) * 128
                                    nc.tensor.matmul(
                                        ps[:, sl],
                                        ceye_t[:, ci:ci + 128],
                                        ht[:, k * N + nchunk * 512:
                                           k * N + (nchunk + 1) * 512],
                                        start=False, stop=(k == KH - 1))
                            pt = PP.tile([128, N], bf16, tag="p", name="p")
                            nc.scalar.activation(pt[:], ps[:], EXP)
                            p_tiles[b].append(pt)
                    for b in range(BPC):
                        pso = PSO.tile([HD + 1, N], f32, tag="pso", name="pso")
                        for mi in range(8):
                            for nchunk in range(2):
                                sl = slice(nchunk * 512, (nchunk + 1) * 512)
                                nc.tensor.matmul(
                                    pso[:, sl],
                                    v_t[b * 8 + mi][:, h * (HD + 1):(h + 1) * (HD + 1)],
                                    p_tiles[b][mi][:, sl],
                                    start=(mi == 0), stop=(mi == 7))
                        # denominator -> broadcast -> reciprocal -> normalize
                        d_t = NRM.tile([1, N], bf16, tag="d", name="d")
                        nc.vector.tensor_copy(d_t[:], pso[64:65, :])
                        R_t = NRM.tile([64, N], f32, tag="R", name="R")
                        for nchunk in range(2):
                            sl = slice(nchunk * 512, (nchunk + 1) * 512)
                            psr = PSM.tile([64, 512], f32, tag="psm", name="psm")
                            nc.tensor.matmul(psr[:], ones_t[:, 0:64], d_t[:, sl],
                                             start=True, stop=True)
                            nc.vector.reciprocal(R_t[:, sl], psr[:])
                        nc.vector.tensor_tensor(
                            oT_t[b * 4 + h // 2][po:po + 64, :],
                            pso[0:64, :], R_t[:], MUL)
                # ---------------- phase 3: output projection ----------------
                for b in range(BPC):
                    for t in range(8):
                        psy = PSM.tile([128, 512], f32, tag="psm", name="psm")
                        for c in range(4):
                            nc.tensor.matmul(
                                psy[:],
                                oT_t[b * 4 + c][:, t * 128:(t + 1) * 128],
                                wp_t[c][:], start=(c == 0), stop=(c == 3))
                        yt = YSB.tile([128, DIM], bf16, tag="y", name="y")
                        nc.vector.tensor_tensor(yt[:], psy[:], bpb_t[:], ADD)
                        nc.sync.dma_start(
                            out=y[b * N + t * 128: b * N + (t + 1) * 128, :],
                            in_=yt[:])
    nc.compile()
    return nc


def _prep_shared(Hstack, hop_logits_attn, rel_alpha, Wqkv, Wproj, bproj):
    """Build the shared blob [SHR_ROWS, 1024] u8 (concat-over-cores layout =
    the flat blob itself, so per-core shards are just row slices)."""
    bf = ml_dtypes.bfloat16
    lg = hop_logits_attn - hop_logits_attn.max(-1, keepdims=True)
    w = np.exp(lg)
    w /= w.sum(-1, keepdims=True)                      # [H, KH]
    # Hstack ships as uint8 (values in [0,1], quantization err ~ bf16's);
    # the 1/255 dequant scale is folded into the scaled identities.
    c_hk = (rel_alpha[:, None] * w).astype(np.float32) / 255.0  # [H, KH]
    eye = np.eye(128, dtype=np.float32)
    ceye = (c_hk.reshape(H * KH, 1, 1) * eye).astype(bf).reshape(CER, 128)
    shr = np.empty((SHR_ROWS, 1024), np.uint8)
    hdst = shr[R_HTS:R_HTS + HTR].reshape(KH, N, N)

    def quant(k):
        np.multiply(Hstack[k].T, 255.0, out=_QBUF[k])
        _QBUF[k] += 0.5
        hdst[k][:] = _QBUF[k]

    list(_POOL.map(quant, range(KH)))
    wqkvT = np.ascontiguousarray(Wqkv.T).astype(np.float32)
    wqkvT[:, :DIM] *= SCALE                            # fold q scaling
    u8row = lambda a: np.ascontiguousarray(a).view(np.uint8).reshape(-1, 1024)
    shr[R_WQKV:R_WPROJ] = u8row(wqkvT.astype(bf))
    shr[R_WPROJ:R_CEYE] = u8row(np.ascontiguousarray(Wproj.T).astype(bf))
    shr[R_CEYE:R_EYE] = u8row(ceye)
    shr[R_EYE:R_BPROJ] = u8row(eye.astype(bf))
    shr[R_BPROJ:R_HTS] = u8row(bproj.astype(np.float32).astype(bf)[None, :])
    shr[R_HTS + HTR:] = 0
    return shr


_QBUF = np.empty((KH, N, N), np.float32)

from concurrent.futures import ThreadPoolExecutor as _TPE
_POOL = _TPE(8)


def _cast_f8(x):
    """Multithreaded f32 -> float8_e4m3 cast (ml_dtypes cast is slow)."""
    out = np.empty(x.shape, ml_dtypes.float8_e4m3)
    chunks = np.array_split(np.arange(x.shape[0]), 8)

    def do(idx):
        out[idx[0]:idx[-1] + 1] = x[idx[0]:idx[-1] + 1]

    list(_POOL.map(do, chunks))
    return out


def _make_runner(nc):
    """Persistent-jit runner for the bass_exec custom call (the axon/PJRT
    path), so warm calls skip tracing and output zero-buffers are created
    on device instead of being shipped from the host."""
    import jax
    import jax.numpy as jnp
    from jax.sharding import Mesh, PartitionSpec, NamedSharding
    from jax.experimental.shard_map import shard_map
    from concourse import mybir
    from concourse.bass2jax import (
        _bass_exec_p, partition_id_tensor, install_neuronx_cc_hook)

    install_neuronx_cc_hook()
    partition_name = nc.partition_id_tensor.name if nc.partition_id_tensor else None
    in_names, out_names, out_avals = [], [], []
    for alloc in nc.m.functions[0].allocations:
        if not isinstance(alloc, mybir.MemoryLocationSet):
            continue
        name = alloc.memorylocations[0].name
        if alloc.kind == "ExternalInput":
            if name != partition_name:
                in_names.append(name)
        elif alloc.kind == "ExternalOutput":
            out_names.append(name)
            out_avals.append(jax.core.ShapedArray(
                tuple(alloc.tensor_shape), mybir.dt.np(alloc.dtype)))
    n_params = len(in_names)
    n_outs = len(out_avals)
    all_names = in_names + out_names
    if partition_name is not None:
        all_names = all_names + [partition_name]
    donate = tuple(range(n_params, n_params + n_outs))

    def _body(*args):
        operands = list(args)
        if partition_name is not None:
            operands.append(partition_id_tensor())
        outs = _bass_exec_p.bind(
            *operands, out_avals=tuple(out_avals), in_names=tuple(all_names),
            out_names=tuple(out_names), lowering_input_output_aliases=(),
            sim_require_finite=True, sim_require_nnan=True, nc=nc)
        return tuple(outs)

    devices = jax.devices()[:NCORES]
    mesh = Mesh(np.asarray(devices), ("core",))
    spec = NamedSharding(mesh, PartitionSpec("core"))
    in_specs = (PartitionSpec("core"),) * (n_params + n_outs)
    out_specs = (PartitionSpec("core"),) * n_outs
    sharded = jax.jit(
        shard_map(_body, mesh=mesh, in_specs=in_specs, out_specs=out_specs,
                  check_rep=False),
        donate_argnums=donate, keep_unused=True)

    zero_shapes = [(NCORES * a.shape[0], *a.shape[1:]) for a in out_avals]
    zero_dtypes = [a.dtype for a in out_avals]
    zeros_fn = jax.jit(
        lambda: tuple(jnp.zeros(s, d) for s, d in zip(zero_shapes, zero_dtypes)),
        out_shardings=tuple(spec for _ in out_avals))

    prev = []

    def run(global_in: dict):
        ins = [global_in[name] for name in in_names]
        # donate the previous call's (already fetched) output buffers as the
        # custom call's result allocation; first call builds zeros on device
        zs = tuple(prev) if prev else zeros_fn()
        prev.clear()
        outs = sharded(*ins, *zs)
        res = {name: np.asarray(o) for name, o in zip(out_names, outs)}
        prev.extend(outs)
        return res

    run.spec = spec
    return run


def kernel(**inputs):
    if "run" not in _CACHE:
        _CACHE["nc"] = _build()
        _CACHE["run"] = _make_runner(_CACHE["nc"])
    import jax
    run = _CACHE["run"]
    # cast + start the async x upload first; build the shared blob while the
    # 8.4MB of fp8 x streams over the tunnel
    x_f8 = _cast_f8(np.asarray(inputs["x"], np.float32).reshape(
        NCORES * TOK, DIM))
    x_dev = jax.device_put(x_f8, run.spec)
    shr = _prep_shared(
        np.asarray(inputs["Hstack"], np.float32),
        np.asarray(inputs["hop_logits_attn"], np.float32),
        np.asarray(inputs["rel_alpha"], np.float32),
        np.asarray(inputs["Wqkv"], np.float32),
        np.asarray(inputs["Wproj"], np.float32),
        np.asarray(inputs["bproj"], np.float32))
    outs = run({"xn": x_dev, "shr_in": shr})
    return outs["y"].astype(np.float32).reshape(B, N, DIM)
